# revision 8
# baseline (speedup 1.0000x reference)
"""Center-contrast triplet loss on 8 Trainium2 NeuronCores — collective-free.

Feature-dim sharding: core m gets the m-th 256-wide feature slice of both
inputs, shipped pre-transposed as [256, 4096] fp16 with the batch axis
reordered to (k, c) so the per-class K-sum is three contiguous halving adds
on the DVE (unit stride). Each core computes partial sum-centers s1/s2, the
partial Gram s1.T @ s2 on TensorE (bf16 operands, f32 PSUM), and the partial
bias rows ss_j = sum_p s2^2 and pp_j = sum_p s1*s2 via ones-matmuls.

No on-device collective: the ncfw rendezvous costs ~75us under this runtime,
dwarfing the 0.5MB of data. Instead every core DMAs its partial Gram
[512, 512] (fp16) plus the two bias rows (f32) straight to its output, and
the host unshard step sums the 8 partials, folds the biases, and runs the
trivial relu/rowmax/cummax/sum epilogue (all values are 32x the true vals
because centers are kept as sums-of-8; the final scalar is divided by 32).

Engine split: Vector owns the K-sum trees and s1*s2 products (it is ~3x
faster than GpSimd on tensor_tensor); Scalar/ACT owns squares and the
PSUM->SBUF casts; loads interleave one chunk of each input per HWDGE queue
so compute streams behind the DMA.
"""

import numpy as np

import concourse.bacc as bacc
import concourse.mybir as mybir
import concourse.tile as tile
from concourse.bass_utils import run_bass_kernel_spmd
from concourse.vector_clock import ScopedClock


class LeanTileContext(tile.TileContext):
    """TileContext with a drain-only exit.

    The stock exit emits drain + all-engine EVSEM barrier + semaphore
    clears + second barrier. The runtime re-arms semaphores at NEFF
    load/execute, so for this single-shot kernel a drain (which already
    waits on every engine's clock) is sufficient; verified correct across
    repeated executions of the same NEFF.
    """

    def _drain_and_barrier(self, tick_clock, wait_clock):
        drain_inst = self.nc.sync.drain()
        wait_clock.add_sem_waits(
            drain_inst.ins, ScopedClock({None: tick_clock.global_clock})
        )
        popped = self.nc._tile_sem_poison_stack.pop()
        assert popped is self._sem_poison
        sems = list(self.sems.allocated().values())
        sem_nums = [s.num if hasattr(s, "num") else s for s in sems]
        self.nc._state.prepend_free_semaphores(sem_nums)
        for poison_set in self.nc._tile_sem_poison_stack:
            poison_set.update(sem_nums)


N_CORES = 8
B, D, C, K = 4096, 2048, 512, 8
DS = D // N_CORES          # 256 features per core
F32 = mybir.dt.float32
F16 = mybir.dt.float16
BF16 = mybir.dt.bfloat16


def build_nc():
    nc = bacc.Bacc(
        "TRN2", target_bir_lowering=False, debug=False, num_devices=N_CORES
    )
    x1t = nc.dram_tensor("x1t", [DS, B], F16, kind="ExternalInput")
    x2t = nc.dram_tensor("x2t", [DS, B], F16, kind="ExternalInput")
    v = nc.dram_tensor("v", [C, C], F16, kind="ExternalOutput")
    ab = nc.dram_tensor("ab", [1, 2 * C], F32, kind="ExternalOutput")

    with LeanTileContext(nc) as tc:
        with (
            tc.tile_pool(name="sbuf", bufs=1) as pool,
            tc.tile_pool(name="psum", bufs=1, space="PSUM") as psum,
        ):
            const_f32 = pool.tile([128, 1], F32, name="const_f32")
            nc.vector.memset(const_f32[:], 1.0)
            ones_col = pool.tile([128, 1], BF16, name="ones_col")
            nc.vector.tensor_copy(ones_col[:], const_f32[:])

            # tiny first DMA warms the HWDGE queue before the big stream
            warm_sb = pool.tile([1, 64], F16, name="warm_sb")
            nc.sync.dma_start(warm_sb[:], x2t[0:1, 0:64])

            # chunk loads: one chunk of EACH input per queue, each chunk as
            # two paired-quarter DMAs (cols q and q+2048 together) so the
            # first K-sum round for a chunk can run at ~50% of its stream
            Q = B // 4
            xq = {}   # (which_input, ch, pair) -> tile [128, 2*Q]
            for (inp, t_dram, ch, eng) in (
                (2, x2t, 0, nc.sync),
                (1, x1t, 0, nc.scalar),
                (2, x2t, 1, nc.sync),
                (1, x1t, 1, nc.scalar),
            ):
                rows = slice(128 * ch, 128 * (ch + 1))
                for p in range(2):
                    t = pool.tile([128, 2 * Q], F16, name=f"x{inp}_{ch}_{p}")
                    eng.dma_start(t[:, 0:Q], t_dram[rows, Q * p : Q * (p + 1)])
                    eng.dma_start(
                        t[:, Q : 2 * Q],
                        t_dram[rows, Q * (p + 2) : Q * (p + 3)],
                    )
                    xq[inp, ch, p] = t

            # contiguous K-sum: batch cols are (k, c) ordered, so summing
            # 8 instances per class is three halving adds at unit stride,
            # all on Vector (GpSimd is ~3x slower on tensor_tensor);
            # round 1 runs as two half-ops, one per paired-quarter load
            def ksum(inp, ch, tag):
                r1 = pool.tile([128, B // 2], F16, name=f"r1_{tag}")
                for p in range(2):
                    t = xq[inp, ch, p]
                    nc.vector.tensor_tensor(
                        r1[:, Q * p : Q * (p + 1)], t[:, 0:Q], t[:, Q : 2 * Q],
                        op=mybir.AluOpType.add,
                    )
                r2 = pool.tile([128, B // 4], F16, name=f"r2_{tag}")
                nc.vector.tensor_tensor(
                    r2[:], r1[:, : B // 4], r1[:, B // 4 :],
                    op=mybir.AluOpType.add,
                )
                s = pool.tile([128, C], BF16, name=f"s_{tag}")
                nc.vector.tensor_tensor(
                    s[:], r2[:, :C], r2[:, C:], op=mybir.AluOpType.add
                )
                return s

            s1, s2, sq, pr = {}, {}, {}, {}
            ss_ps = psum.tile([1, C], F32, name="ss_ps")
            pp_ps = psum.tile([1, C], F32, name="pp_ps")
            g_ps = [
                psum.tile([128, C], F32, name=f"g{q}", tag="gps", bufs=4)
                for q in range(4)
            ]
            with nc.allow_low_precision(reason="16-bit tree-sum for centers"):
                for ch in range(2):
                    s2[ch] = ksum(2, ch, f"x2_{ch}")
                    s1[ch] = ksum(1, ch, f"x1_{ch}")
                    # squares on ACT, products on Vector
                    sq[ch] = pool.tile([128, C], BF16, name=f"sq{ch}")
                    nc.scalar.square(sq[ch][:], s2[ch][:])
                    pr[ch] = pool.tile([128, C], BF16, name=f"pr{ch}")
                    nc.vector.tensor_mul(pr[ch][:], s1[ch][:], s2[ch][:])
                    # partial Gram row-blocks for this chunk
                    for q in range(4):
                        cs = slice(128 * q, 128 * (q + 1))
                        nc.tensor.matmul(
                            g_ps[q][:], lhsT=s1[ch][:, cs], rhs=s2[ch][:],
                            start=(ch == 0), stop=(ch == 1),
                        )
                    nc.tensor.matmul(
                        ss_ps[:], lhsT=ones_col[:], rhs=sq[ch][:],
                        start=(ch == 0), stop=(ch == 1),
                    )
                    nc.tensor.matmul(
                        pp_ps[:], lhsT=ones_col[:], rhs=pr[ch][:],
                        start=(ch == 0), stop=(ch == 1),
                    )

            # PSUM -> SBUF casts on ACT, DMAs spread over both HWDGE queues
            out_eng = [nc.sync, nc.scalar, nc.sync, nc.scalar]
            for q in range(4):
                v_sb = pool.tile([128, C], F16, name=f"v_sb{q}")
                nc.scalar.copy(v_sb[:], g_ps[q][:])
                out_eng[q].dma_start(v[128 * q : 128 * (q + 1), :], v_sb[:])

            ab_sb = pool.tile([1, 2 * C], F32, name="ab_sb")
            nc.vector.tensor_copy(ab_sb[:, 0:C], ss_ps[:])
            nc.vector.tensor_copy(ab_sb[:, C : 2 * C], pp_ps[:])
            nc.gpsimd.dma_start(ab[:], ab_sb[:])

    nc.finalize()
    return nc


def prepare_in_maps(input1, input2):
    x1 = np.asarray(input1, dtype=np.float32)
    x2 = np.asarray(input2, dtype=np.float32)
    # [D, B] with batch reordered from (c, k) to (k, c): one big strided
    # gather per input, then per-core slices are contiguous views
    x1t = np.ascontiguousarray(
        x1.T.reshape(D, C, K).transpose(0, 2, 1), dtype=np.float16
    ).reshape(D, B)
    x2t = np.ascontiguousarray(
        x2.T.reshape(D, C, K).transpose(0, 2, 1), dtype=np.float16
    ).reshape(D, B)
    in_maps = []
    for m in range(N_CORES):
        sl = slice(m * DS, (m + 1) * DS)
        in_maps.append({"x1t": x1t[sl], "x2t": x2t[sl]})
    return in_maps


def postprocess(results):
    g = np.zeros((C, C), dtype=np.float32)
    ss = np.zeros(C, dtype=np.float64)
    pp = np.zeros(C, dtype=np.float64)
    for m in range(N_CORES):
        g += np.asarray(results[m]["v"], dtype=np.float32)
        a = np.asarray(results[m]["ab"], dtype=np.float64).reshape(2 * C)
        ss += a[:C]
        pp += a[C:]
    a_col = 0.5 * ss - pp          # per-row bias
    b_row = 0.5 * ss               # per-col bias
    vfull = g + (a_col[:, None] - b_row[None, :]).astype(np.float32)
    rm = np.maximum(vfull.max(axis=1), 0.0) / 32.0
    return np.float32(np.maximum.accumulate(rm).sum())


_NC_CACHE = None


def kernel(input1, input2, targets1, targets2):
    global _NC_CACHE
    if _NC_CACHE is None:
        _NC_CACHE = build_nc()
    in_maps = prepare_in_maps(input1, input2)
    res = run_bass_kernel_spmd(_NC_CACHE, in_maps, list(range(N_CORES)))
    return postprocess(res.results)


# revision 9
# speedup vs baseline: 1.1340x; 1.1340x over previous
"""Center-contrast triplet loss on 8 Trainium2 NeuronCores — collective-free.

Feature-dim sharding: core m gets the m-th 256-wide feature slice of both
inputs, shipped pre-transposed as [256, 4096] fp16 with the batch axis
reordered to (k, c) so the per-class K-sum is three contiguous halving adds
on the DVE (unit stride). Each core computes partial sum-centers s1/s2, the
partial Gram s1.T @ s2 on TensorE (bf16 operands, f32 PSUM), and the partial
bias rows ss_j = sum_p s2^2 and pp_j = sum_p s1*s2 via ones-matmuls.

No on-device collective: the ncfw rendezvous costs ~75us under this runtime,
dwarfing the 0.5MB of data. Instead every core DMAs its partial Gram
[512, 512] (fp16) plus the two bias rows (f32) straight to its output, and
the host unshard step sums the 8 partials, folds the biases, and runs the
trivial relu/rowmax/cummax/sum epilogue (all values are 32x the true vals
because centers are kept as sums-of-8; the final scalar is divided by 32).

Engine split: Vector owns the K-sum trees and s1*s2 products (it is ~3x
faster than GpSimd on tensor_tensor); Scalar/ACT owns squares and the
PSUM->SBUF casts; loads interleave one chunk of each input per HWDGE queue
so compute streams behind the DMA.
"""

import numpy as np

import concourse.bacc as bacc
import concourse.mybir as mybir
import concourse.tile as tile
from concourse.bass_utils import run_bass_kernel_spmd
from concourse.vector_clock import ScopedClock


class LeanTileContext(tile.TileContext):
    """TileContext with a drain-only exit.

    The stock exit emits drain + all-engine EVSEM barrier + semaphore
    clears + second barrier. The runtime re-arms semaphores at NEFF
    load/execute, so for this single-shot kernel a drain (which already
    waits on every engine's clock) is sufficient; verified correct across
    repeated executions of the same NEFF.
    """

    def _drain_and_barrier(self, tick_clock, wait_clock):
        drain_inst = self.nc.sync.drain()
        wait_clock.add_sem_waits(
            drain_inst.ins, ScopedClock({None: tick_clock.global_clock})
        )
        popped = self.nc._tile_sem_poison_stack.pop()
        assert popped is self._sem_poison
        sems = list(self.sems.allocated().values())
        sem_nums = [s.num if hasattr(s, "num") else s for s in sems]
        self.nc._state.prepend_free_semaphores(sem_nums)
        for poison_set in self.nc._tile_sem_poison_stack:
            poison_set.update(sem_nums)


N_CORES = 8
B, D, C, K = 4096, 2048, 512, 8
DS = D // N_CORES          # 256 features per core
F32 = mybir.dt.float32
F16 = mybir.dt.float16
BF16 = mybir.dt.bfloat16


def build_nc():
    nc = bacc.Bacc(
        "TRN2", target_bir_lowering=False, debug=False, num_devices=N_CORES
    )
    x1t = nc.dram_tensor("x1t", [DS, B], F16, kind="ExternalInput")
    x2t = nc.dram_tensor("x2t", [DS, B], F16, kind="ExternalInput")
    v = nc.dram_tensor("v", [C, C], F16, kind="ExternalOutput")
    ab = nc.dram_tensor("ab", [1, 2 * C], F32, kind="ExternalOutput")

    with LeanTileContext(nc) as tc:
        with (
            tc.tile_pool(name="sbuf", bufs=1) as pool,
            tc.tile_pool(name="psum", bufs=1, space="PSUM") as psum,
        ):
            const_f32 = pool.tile([128, 1], F32, name="const_f32")
            nc.vector.memset(const_f32[:], 1.0)
            ones_col = pool.tile([128, 1], BF16, name="ones_col")
            nc.vector.tensor_copy(ones_col[:], const_f32[:])

            # tiny first DMA warms the HWDGE queue before the big stream
            warm_sb = pool.tile([1, 64], F16, name="warm_sb")
            nc.sync.dma_start(warm_sb[:], x2t[0:1, 0:64])

            # chunk loads: one chunk of EACH input per queue, so both an s2
            # and an s1 chunk land early and compute can start at ~50% of
            # the stream
            xs = {}   # (which_input, ch) -> tile
            for (inp, t_dram, ch, eng) in (
                (2, x2t, 0, nc.sync),
                (1, x1t, 0, nc.scalar),
                (2, x2t, 1, nc.sync),
                (1, x1t, 1, nc.scalar),
            ):
                t = pool.tile([128, B], F16, name=f"x{inp}_{ch}")
                eng.dma_start(t[:], t_dram[128 * ch : 128 * (ch + 1), :])
                xs[inp, ch] = t

            # contiguous K-sum: batch cols are (k, c) ordered, so summing
            # 8 instances per class is three halving adds at unit stride,
            # all on Vector (GpSimd is ~3x slower on tensor_tensor)
            def ksum(src, tag):
                r1 = pool.tile([128, B // 2], F16, name=f"r1_{tag}")
                nc.vector.tensor_tensor(
                    r1[:], src[:, : B // 2], src[:, B // 2 :],
                    op=mybir.AluOpType.add,
                )
                r2 = pool.tile([128, B // 4], F16, name=f"r2_{tag}")
                nc.vector.tensor_tensor(
                    r2[:], r1[:, : B // 4], r1[:, B // 4 :],
                    op=mybir.AluOpType.add,
                )
                s = pool.tile([128, C], BF16, name=f"s_{tag}")
                nc.vector.tensor_tensor(
                    s[:], r2[:, :C], r2[:, C:], op=mybir.AluOpType.add
                )
                return s

            s1, s2, sq, pr = {}, {}, {}, {}
            ss_ps = psum.tile([1, C], F32, name="ss_ps")
            pp_ps = psum.tile([1, C], F32, name="pp_ps")
            g_ps = [
                psum.tile([128, C], F32, name=f"g{q}", tag="gps", bufs=4)
                for q in range(4)
            ]
            with nc.allow_low_precision(reason="16-bit tree-sum for centers"):
                for ch in range(2):
                    s2[ch] = ksum(xs[2, ch], f"x2_{ch}")
                    s1[ch] = ksum(xs[1, ch], f"x1_{ch}")
                    # squares on ACT, products on Vector
                    sq[ch] = pool.tile([128, C], BF16, name=f"sq{ch}")
                    nc.scalar.square(sq[ch][:], s2[ch][:])
                    pr[ch] = pool.tile([128, C], BF16, name=f"pr{ch}")
                    nc.vector.tensor_mul(pr[ch][:], s1[ch][:], s2[ch][:])
                    # partial Gram row-blocks for this chunk
                    for q in range(4):
                        cs = slice(128 * q, 128 * (q + 1))
                        nc.tensor.matmul(
                            g_ps[q][:], lhsT=s1[ch][:, cs], rhs=s2[ch][:],
                            start=(ch == 0), stop=(ch == 1),
                        )
                    nc.tensor.matmul(
                        ss_ps[:], lhsT=ones_col[:], rhs=sq[ch][:],
                        start=(ch == 0), stop=(ch == 1),
                    )
                    nc.tensor.matmul(
                        pp_ps[:], lhsT=ones_col[:], rhs=pr[ch][:],
                        start=(ch == 0), stop=(ch == 1),
                    )

            # PSUM -> SBUF casts on ACT, DMAs spread over both HWDGE queues
            out_eng = [nc.sync, nc.scalar, nc.sync, nc.scalar]
            for q in range(4):
                v_sb = pool.tile([128, C], F16, name=f"v_sb{q}")
                nc.scalar.copy(v_sb[:], g_ps[q][:])
                out_eng[q].dma_start(v[128 * q : 128 * (q + 1), :], v_sb[:])

            ab_sb = pool.tile([1, 2 * C], F32, name="ab_sb")
            nc.vector.tensor_copy(ab_sb[:, 0:C], ss_ps[:])
            nc.vector.tensor_copy(ab_sb[:, C : 2 * C], pp_ps[:])
            nc.gpsimd.dma_start(ab[:], ab_sb[:])

    nc.finalize()
    return nc


def prepare_in_maps(input1, input2):
    x1 = np.asarray(input1, dtype=np.float32)
    x2 = np.asarray(input2, dtype=np.float32)
    # [D, B] with batch reordered from (c, k) to (k, c): one big strided
    # gather per input, then per-core slices are contiguous views
    x1t = np.ascontiguousarray(
        x1.T.reshape(D, C, K).transpose(0, 2, 1), dtype=np.float16
    ).reshape(D, B)
    x2t = np.ascontiguousarray(
        x2.T.reshape(D, C, K).transpose(0, 2, 1), dtype=np.float16
    ).reshape(D, B)
    in_maps = []
    for m in range(N_CORES):
        sl = slice(m * DS, (m + 1) * DS)
        in_maps.append({"x1t": x1t[sl], "x2t": x2t[sl]})
    return in_maps


def postprocess(results):
    g = np.zeros((C, C), dtype=np.float32)
    ss = np.zeros(C, dtype=np.float64)
    pp = np.zeros(C, dtype=np.float64)
    for m in range(N_CORES):
        g += np.asarray(results[m]["v"], dtype=np.float32)
        a = np.asarray(results[m]["ab"], dtype=np.float64).reshape(2 * C)
        ss += a[:C]
        pp += a[C:]
    a_col = 0.5 * ss - pp          # per-row bias
    b_row = 0.5 * ss               # per-col bias
    vfull = g + (a_col[:, None] - b_row[None, :]).astype(np.float32)
    rm = np.maximum(vfull.max(axis=1), 0.0) / 32.0
    return np.float32(np.maximum.accumulate(rm).sum())


_NC_CACHE = None


def kernel(input1, input2, targets1, targets2):
    global _NC_CACHE
    if _NC_CACHE is None:
        _NC_CACHE = build_nc()
    in_maps = prepare_in_maps(input1, input2)
    res = run_bass_kernel_spmd(_NC_CACHE, in_maps, list(range(N_CORES)))
    return postprocess(res.results)
